# revision 25
# baseline (speedup 1.0000x reference)
"""Multi-head causal attention (B=2, S=2048, HID=2048, H=16, D=128) on 8 TRN2
NeuronCores.

Sharding: core c handles batch b=c//4 and heads [4*(c%4) .. 4*(c%4)+3].
Each core computes qkv-projection + RoPE + causal attention + its partial
out-projection; the host sums the 4 partial outputs per batch (tensor-parallel
reduce) and stacks the 2 batches.

Layout/perf notes:
  - all matmul operands are bf16 (PSUM accumulation stays fp32): same PE
    streaming rate as fp32r but FWL makes LDWEIGHTS ~4x faster and halves
    DMA/SBUF.  All activations stay transposed ([feature, token]) so the
    whole chain runs on the PE with no transposes; RoPE rotate-half is two
    SBUF-SBUF partition-swap DMAs with the sign folded into the sin table.
  - INTERLEAVED EMISSION: Tile's scheduler priority follows program order,
    so projection, attention and out-projection work units are emitted
    round-robin (attention over token blocks <= jb while block jb+1
    projects; out-projection of q-block g while later attention runs).
    This spreads the softmax exp (the only ScalarE-bound stage) across the
    whole kernel instead of bunching it after projection.
  - attention processes one 128-k-chunk at a time ([128,512] scores in one
    PSUM bank).  Causal trim: chunk kc only computes q >= 128*kc; the exp
    writes the same trimmed range into persistent pre-zeroed tiles so the
    masked-dead region is exactly 0 (exp never reads never-written PSUM,
    which can hold Inf/NaN boot garbage).
  - softmax denominator: exp'd chunks are pair+quad-summed on DVE, one
    ones-matmul per 4 chunks on PE; 1/R is the fast DVE reciprocal approx.
  - PSUM budget is exactly 8 banks: P(2) + Pv/P3 shared tag(2) + Sc(2) +
    O(1) + R(1).
"""
import sys

sys.path.insert(0, '/opt/trn_rl_repo')

import numpy as np

B, S, HID = 2, 2048, 2048
H, D = 16, 128
NH = H // 4          # heads per core = 4
HC = HID // 128      # hid chunks = 16
TB = 512             # token block for projection
NTB = S // TB        # 4
QB = 512             # q block in attention
NQB = S // QB        # 4
NKCH = S // 128      # k chunks total = 16
SCALE = 1.0 / float(np.sqrt(D))
BASE = 10000.0
N_CORES = 8

_cache = {}


def _build():
    import concourse.bass as bass  # noqa: F401
    import concourse.tile as tile
    from concourse import bacc, mybir

    f32 = mybir.dt.float32
    bf16 = mybir.dt.bfloat16
    EXP = mybir.ActivationFunctionType.Exp
    MULT = mybir.AluOpType.mult
    ADD = mybir.AluOpType.add

    nc = bacc.Bacc("TRN2", target_bir_lowering=False, debug=False,
                   num_devices=N_CORES)

    xT = nc.dram_tensor("xT", [HID, S], bf16, kind="ExternalInput").ap()
    wqk = nc.dram_tensor("wqk", [HID, 2 * NH * D], bf16, kind="ExternalInput").ap()
    wv = nc.dram_tensor("wv", [HID, NH * D], bf16, kind="ExternalInput").ap()
    wo = nc.dram_tensor("wo", [NH * D, HID], bf16, kind="ExternalInput").ap()
    cosT = nc.dram_tensor("cosT", [D, S], bf16, kind="ExternalInput").ap()
    sinS = nc.dram_tensor("sinS", [D, S], bf16, kind="ExternalInput").ap()
    maskT = nc.dram_tensor("maskT", [128, 4 * QB], bf16, kind="ExternalInput").ap()
    ones_sq = nc.dram_tensor("ones_sq", [128, 128], bf16, kind="ExternalInput").ap()
    y = nc.dram_tensor("y", [S, HID], bf16, kind="ExternalOutput").ap()

    with tile.TileContext(nc) as tc, \
         tc.tile_pool(name="persist", bufs=1) as pp, \
         tc.tile_pool(name="p2", bufs=4) as p2, \
         tc.tile_pool(name="p2c", bufs=1) as p2c, \
         tc.tile_pool(name="p2r", bufs=2) as p2r, \
         tc.tile_pool(name="p23w", bufs=1) as p2w, \
         tc.tile_pool(name="w1", bufs=1) as w1p, \
         tc.tile_pool(name="x1", bufs=2) as x1p, \
         tc.tile_pool(name="trig", bufs=1) as tgp, \
         tc.tile_pool(name="rope", bufs=2) as rp, \
         tc.tile_pool(name="p3", bufs=4) as p3, \
         tc.tile_pool(name="ps1", bufs=2, space="PSUM") as ps1, \
         tc.tile_pool(name="psv", bufs=2, space="PSUM") as psv, \
         tc.tile_pool(name="psS", bufs=2, space="PSUM") as psS, \
         tc.tile_pool(name="psO", bufs=1, space="PSUM") as psO, \
         tc.tile_pool(name="psR", bufs=1, space="PSUM") as psR:

        # per-512-block tiles so consumers can start before phase 1 ends
        qkT = [[pp.tile([128, TB], bf16, tag=f"qkT{i}_{j}",
                        name=f"qkT{i}_{j}") for j in range(NTB)]
               for i in range(8)]
        v_all = [pp.tile([128, NH * D], bf16, tag=f"v{cg}", name=f"v{cg}")
                 for cg in range(NKCH)]
        tmask = p2c.tile([128, 4 * QB], bf16, tag="tmask")
        t1s = p2c.tile([128, 128], bf16, tag="t1s")
        # persistent exp-output tiles for diagonal chunk kinds 1..3: only
        # [128*dk : 512) is ever written by exp, so [0 : 128*dk) stays the
        # memset zero forever and the masked dead region is exactly 0.
        at_t = [p2c.tile([128, QB], bf16, tag=f"at{dk}", name=f"at{dk}")
                for dk in range(1, 4)]
        for t_ in at_t:
            nc.gpsimd.memset(t_[:], 0.0)
        wot = [p2w.tile([128, HID], bf16, tag=f"wot{h}", name=f"wot{h}")
               for h in range(NH)]
        outT = [[p2w.tile([128, QB], bf16, tag=f"outT{h}_{j}",
                          name=f"outT{h}_{j}") for j in range(NQB)]
                for h in range(NH)]

        # ---------- DMAs (order = arrival priority) ----------
        wvt = w1p.tile([128, HC * NH * D], bf16, tag="wvt")
        xTbs = [x1p.tile([128, HC * TB], bf16, tag="xTb", name=f"xTb{j}")
                for j in range(NTB)]
        for q4 in range(4):
            cs = slice(q4 * 4, (q4 + 1) * 4)
            nc.sync.dma_start(
                wvt[:].rearrange("p (c n) -> p c n", n=NH * D)[:, cs],
                wv.rearrange("(c p) n -> p c n", p=128)[:, cs])
            nc.sync.dma_start(
                xTbs[0][:].rearrange("p (c t) -> p c t", t=TB)[:, cs],
                xT.rearrange("(c p) t -> p c t", p=128)[:, cs, 0:TB])
        # x block 1 before the qk weights: the rot-swap SBUF DMAs of window 0
        # share the queues, so anything emitted later slips ~10us.
        nc.sync.dma_start(
            xTbs[1][:].rearrange("p (c t) -> p c t", t=TB),
            xT.rearrange("(c p) t -> p c t", p=128)[:, :, TB:2 * TB])
        wqkt = w1p.tile([128, HC * 2 * NH * D], bf16, tag="wqkt")
        for cc in range(8):
            ns_ = slice(cc * 128, (cc + 1) * 128)
            nc.sync.dma_start(
                wqkt[:].rearrange("p (c n) -> p c n", n=2 * NH * D)[:, :, ns_],
                wqk.rearrange("(c p) n -> p c n", p=128)[:, :, ns_])
        tcos = tgp.tile([D, S], bf16, tag="tcos")
        tsin = tgp.tile([D, S], bf16, tag="tsin")
        nc.sync.dma_start(tcos[:], cosT)
        nc.sync.dma_start(tsin[:], sinS)
        nc.sync.dma_start(tmask[:], maskT[:])
        nc.sync.dma_start(t1s[:], ones_sq[:])
        for h in range(NH):
            nc.sync.dma_start(wot[h][:], wo[h * 128:(h + 1) * 128, :])

        # ---------- work-unit generators ----------
        def proj_units(jb):
            """12 units: x DMA + 4 V-chunks + 8 QK columns (with RoPE)."""
            xTb = xTbs[jb]
            if jb > 1:  # blocks 0 and 1 are DMA'd up front; quarter-split
                for q4 in range(4):  # so V-proj starts on the first quarter
                    cs = slice(q4 * 4, (q4 + 1) * 4)
                    nc.sync.dma_start(
                        xTb[:].rearrange("p (c t) -> p c t", t=TB)[:, cs],
                        xT.rearrange("(c p) t -> p c t", p=128)[
                            :, cs, jb * TB:(jb + 1) * TB])
            for t2 in range(TB // 128):
                cg = jb * (TB // 128) + t2
                Pv = psv.tile([128, NH * D], f32, tag="Pv", name=f"Pv{cg}")
                for c in range(HC):
                    nc.tensor.matmul(
                        Pv[:],
                        xTb[:, c * TB + t2 * 128: c * TB + (t2 + 1) * 128],
                        wvt[:, c * NH * D:(c + 1) * NH * D],
                        start=(c == 0), stop=(c == HC - 1))
                nc.scalar.copy(v_all[cg][:], Pv[:])
                yield
            sl = slice(jb * TB, (jb + 1) * TB)
            for cc in range(8):
                P = ps1.tile([128, TB], f32, tag="P")
                for c in range(HC):
                    nc.tensor.matmul(
                        P[:],
                        wqkt[:, c * 2 * NH * D + cc * 128:
                             c * 2 * NH * D + (cc + 1) * 128],
                        xTb[:, c * TB:(c + 1) * TB],
                        start=(c == 0), stop=(c == HC - 1))
                u = rp.tile([128, TB], f32, tag="u")
                nc.vector.tensor_copy(u[:], P[:])
                rot = rp.tile([128, TB], f32, tag="rot")
                nc.sync.dma_start(rot[0:64, :], u[64:128, :])
                nc.sync.dma_start(rot[64:128, :], u[0:64, :])
                m = rp.tile([128, TB], f32, tag="m")
                nc.vector.tensor_tensor(
                    out=m[:], in0=rot[:], in1=tsin[:, sl], op=MULT)
                t = rp.tile([128, TB], f32, tag="rot", name="t")
                nc.vector.tensor_tensor(
                    out=t[:], in0=u[:], in1=tcos[:, sl], op=MULT)
                nc.vector.tensor_tensor(
                    out=qkT[cc][jb][:], in0=t[:], in1=m[:], op=ADD)
                yield

        def attn_units(jb4):
            """One unit per k-chunk (plus head epilogues)."""
            nkc = (QB // 128) * (jb4 + 1)
            for h in range(NH):
                O = psO.tile([128, QB], f32, tag="O")
                R = psR.tile([128, QB], f32, tag="R")
                pair_hold = None
                a01_prev = None
                aq_run = None
                for kc in range(nkc):
                    dk = kc - 4 * jb4          # diagonal kind 0..3, else <0
                    off = max(0, 128 * dk)     # causal trim: q >= 128*kc
                    Sc = psS.tile([128, QB], f32, tag="S")
                    nc.tensor.matmul(
                        Sc[:, off:QB],
                        qkT[NH + h][kc // 4][
                            :, (kc % 4) * 128:(kc % 4 + 1) * 128],
                        qkT[h][jb4][:, off:QB], start=True, stop=True)
                    if dk >= 1:
                        At = at_t[dk - 1]
                        nc.scalar.activation(At[:, off:QB], Sc[:, off:QB],
                                             EXP, scale=SCALE)
                        A = p2.tile([128, QB], bf16, tag="A", bufs=4)
                        nc.vector.tensor_tensor(
                            out=A[:], in0=At[:],
                            in1=tmask[:, dk * QB:(dk + 1) * QB], op=MULT)
                    elif dk == 0:
                        Ae = p2.tile([128, QB], bf16, tag="Ae", bufs=1)
                        nc.scalar.activation(Ae[:], Sc[:], EXP, scale=SCALE)
                        A = p2.tile([128, QB], bf16, tag="A", bufs=4)
                        nc.vector.tensor_tensor(
                            out=A[:], in0=Ae[:], in1=tmask[:, 0:QB], op=MULT)
                    else:
                        A = p2.tile([128, QB], bf16, tag="A", bufs=4)
                        nc.scalar.activation(A[:], Sc[:], EXP, scale=SCALE)
                    nc.tensor.matmul(
                        O[:, off:QB], v_all[kc][:, h * D:(h + 1) * D],
                        A[:, off:QB],
                        start=(kc == 0), stop=(kc == nkc - 1))
                    if kc % 2 == 0:
                        pair_hold = A
                    else:
                        A01 = p2.tile([128, QB], bf16, tag="A01", bufs=2)
                        nc.vector.tensor_tensor(
                            out=A01[:], in0=pair_hold[:], in1=A[:], op=ADD)
                        if (kc // 2) % 2 == 0:
                            a01_prev = A01
                        else:   # quad boundary: fold into the running sum
                            Aq = p2.tile([128, QB], bf16, tag="Aq", bufs=2)
                            nc.vector.tensor_tensor(
                                out=Aq[:], in0=a01_prev[:], in1=A01[:], op=ADD)
                            if aq_run is None:
                                aq_run = Aq
                            else:
                                Aq8 = p2.tile([128, QB], bf16, tag="Aq8",
                                              bufs=2)
                                nc.vector.tensor_tensor(
                                    out=Aq8[:], in0=aq_run[:], in1=Aq[:],
                                    op=ADD)
                                aq_run = Aq8
                    yield
                nc.tensor.matmul(R[:], t1s[:], aq_run[:],
                                 start=True, stop=True)
                rec = p2r.tile([128, QB], f32, tag="rec")
                nc.vector.reciprocal_approx_fast(rec[:], R[:])
                nc.vector.tensor_tensor(
                    out=outT[h][jb4][:], in0=O[:], in1=rec[:], op=MULT)
                yield

        def ph3_units(g):
            """16 units: out-projection for token chunks [4g, 4g+4).
            Groups >= 2 run after projection is done, so they can alternate
            between the Pv and P PSUM bank-pairs (4-deep pipeline) instead
            of contending for Pv's 2 banks."""
            for un, tch in enumerate(range(4 * g, 4 * g + 4)):
                for cb in range(HID // 512):
                    ptag = "Pv" if (g < 2 or (un * 4 + cb) % 2 == 0) else "P"
                    pool = psv if ptag == "Pv" else ps1
                    P3 = pool.tile([128, 512], f32, tag=ptag,
                                   name=f"P3_{tch}_{cb}")
                    for h in range(NH):
                        nc.tensor.matmul(
                            P3[:],
                            outT[h][tch // 4][
                                :, (tch % 4) * 128:(tch % 4 + 1) * 128],
                            wot[h][:, cb * 512:(cb + 1) * 512],
                            start=(h == 0), stop=(h == NH - 1))
                    ys = p3.tile([128, 512], bf16,
                                 tag="ys" if g < 2 else "ys2",
                                 bufs=2 if g < 2 else 3)
                    nc.vector.tensor_copy(ys[:], P3[:])
                    nc.sync.dma_start(
                        y[tch * 128:(tch + 1) * 128,
                          cb * 512:(cb + 1) * 512], ys[:])
                    yield

        def run_merged(parts):
            """parts: list of (generator, n_units). Emits all units,
            interleaved proportionally; earlier parts lead on ties."""
            gens = [(g, n) for g, n in parts if n > 0]
            prog = [0.0] * len(gens)
            remaining = [n for _, n in gens]
            while any(r > 0 for r in remaining):
                best, best_v = -1, None
                for i, ((_, n), r) in enumerate(zip(gens, remaining)):
                    if r <= 0:
                        continue
                    v = prog[i] / n
                    if best_v is None or v < best_v - 1e-12:
                        best, best_v = i, v
                next(gens[best][0])
                prog[best] += 1.0
                remaining[best] -= 1

        # ---------- merged schedule ----------
        NA = [NH * ((QB // 128) * (j + 1) + 1) for j in range(NQB)]  # units
        run_merged([(proj_units(0), 12)])
        run_merged([(proj_units(1), 12), (attn_units(0), NA[0])])
        run_merged([(proj_units(2), 12), (attn_units(1), NA[1]),
                    (ph3_units(0), 16)])
        run_merged([(proj_units(3), 12), (attn_units(2), NA[2]),
                    (ph3_units(1), 16)])
        run_merged([(attn_units(3), NA[3]), (ph3_units(2), 16)])
        run_merged([(ph3_units(3), 16)])

    nc.compile()
    return nc


def _host_inputs(x, w_qkv, w_out):
    """Build the 8 per-core input maps."""
    import ml_dtypes
    bf = ml_dtypes.bfloat16

    # RoPE tables, transposed ([d, t]) with the rotate-half sign folded in.
    inv_freq = 1.0 / (BASE ** (np.arange(0, D, 2, dtype=np.float64) / D))
    pos = np.arange(S, dtype=np.float64)
    freqs = np.outer(inv_freq, pos)           # [64, S]
    cos_h = np.cos(freqs).astype(np.float32)
    sin_h = np.sin(freqs).astype(np.float32)
    cosT = np.concatenate([cos_h, cos_h], 0).astype(bf)   # [128, S]
    sinS = np.concatenate([-sin_h, sin_h], 0).astype(bf)  # signed sin

    # Causal masks for the 4 diagonal sub-blocks ([k-part, q-free])
    kp = np.arange(128)[:, None]
    qf = np.arange(QB)[None, :]
    maskT = np.concatenate(
        [(qf >= 128 * mm + kp).astype(bf) for mm in range(4)], axis=1)

    w3 = np.asarray(w_qkv, np.float32).reshape(HID, 3, H, D)
    wo_full = np.asarray(w_out, np.float32).reshape(H, D, HID)
    x = np.asarray(x, np.float32)

    shared = {
        "cosT": cosT, "sinS": sinS, "maskT": maskT,
        "ones_sq": np.ones((128, 128), bf),
    }
    in_maps = []
    for c in range(N_CORES):
        b, hg = c // 4, c % 4
        heads = slice(4 * hg, 4 * hg + 4)
        wqk = np.ascontiguousarray(
            w3[:, 0:2, heads, :].reshape(HID, 2 * NH * D)).astype(bf)
        wv = np.ascontiguousarray(
            w3[:, 2, heads, :].reshape(HID, NH * D)).astype(bf)
        wo_c = np.ascontiguousarray(wo_full[heads].reshape(NH * D, HID)).astype(bf)
        in_maps.append({
            "xT": np.ascontiguousarray(x[b].T).astype(bf),
            "wqk": wqk, "wv": wv, "wo": wo_c, **shared,
        })
    return in_maps


def kernel(x, w_qkv, w_out):
    from concourse.bass_utils import run_bass_kernel_spmd

    if "nc" not in _cache:
        _cache["nc"] = _build()
    nc = _cache["nc"]
    in_maps = _host_inputs(x, w_qkv, w_out)
    res = run_bass_kernel_spmd(nc, in_maps, core_ids=list(range(N_CORES)))
    out = np.zeros((B, S, HID), np.float32)
    for c in range(N_CORES):
        out[c // 4] += np.asarray(res.results[c]["y"], np.float32)
    return out
